# revision 17
# baseline (speedup 1.0000x reference)
"""Trainium2 Bass kernel for nn_CausalSelfAttention_42623255446168.

Contract: kernel(**inputs) takes FULL unsharded inputs (as produced by
setup_inputs()) and returns the FULL output [1, 2048, 1024] float32.

Sharding: tensor-parallel over the 16 query heads across 8 cores
(2 q-heads / core; each core uses exactly one GQA kv-head = core//2).
Each core computes a partial output projection [2048, 1024]; the host
sums the 8 partials (the "all-reduce" of the hint, done host-side).

Per-core device pipeline (layouts transposed: dims on partitions,
sequence on the free axis; PE matmuls in f32r/tf32):
  1. qT/kTd/vgT projections from xT; weights pre-transposed on host; k is
     duplicated [k1;k2;k1;k2] so the 4 differential-attention sub-units
     (2 heads x 2 halves) occupy distinct 32-partition groups.
  2. RMS-norm folded as per-(head,pos) scales via PE mask-matmuls
     (sum of squares), sqrt+reciprocal, partition re-broadcast with tiny
     PE matmuls. q-gain and 1/sqrt(32) folded into the q scale.
  3. Rotary in transposed layout: partition-swap via a PE permutation
     matmul + elementwise muls with host cos/sin tables (rotary is
     linear, so applying it after the norm scale is exact).
  4. Value gate sigmoid+mul, PE transposes of v to natural layout with
     interleaved ones columns (v_aug) so each PV matmul also produces
     the softmax denominator row for free.
  5. Attention per 512-wide q-super-block: QK^T scores for the 4
     sub-units in one 4-bank PSUM arena (tile_position row packing,
     K=32), causal-diagonal masking by accumulating a bf16 upper-tri
     -1e9 matmul into the scores, one fused Exp over all 4 banks (no
     max-subtraction: RMS-normed scores are bounded by ~11.4), PV
     accumulation (K=128, M=33, two sub-units column-packed per bank).
  6. Normalization via PE-only partition moves: reciprocal of the
     denominator row, K=1 broadcast matmul, multiply, then a placement
     matmul assembling the [128, S] uT tile.
  7. Output projection against host-prepared WoP (lambda and the
     differential +/- concat folded in), PSUM->SBUF copy, DMA out.
"""

import functools
import math
import os
import sys

import numpy as np

sys.path.insert(0, "/opt/trn_rl_repo")

import ml_dtypes  # noqa: E402

import concourse.bass as bass  # noqa: E402
import concourse.tile as tile  # noqa: E402
from concourse import bacc, mybir  # noqa: E402
from concourse.bass_utils import run_bass_kernel_spmd  # noqa: E402

S = 2048
DIM = 1024
H = 16
KVH = 4
HD = 64
HALF = 32
NCORES = 8
EPS = float(np.finfo(np.float32).eps)

F32 = mybir.dt.float32
F32R = mybir.dt.float32r
BF16 = mybir.dt.bfloat16
DEBUG = bool(int(os.environ.get("KBG_DEBUG", "0")))
PSUM = bass.MemorySpace.PSUM

QS = 4    # q-super blocks
QW = 512  # q-super width


def _r(ap):
    """Bitcast an f32 AP to f32r for full-rate PE matmuls."""
    return ap.bitcast(F32R)


def _build_kernel(tc, io):
    nc = tc.nc

    with tc.tile_pool(name="persist", bufs=1) as persist:
        # per-super-chunk tiles so phase-D consumption can overlap
        # phase-C production chunk by chunk
        qRc = [persist.tile([128, QW], F32, name=f"qRc{i}") for i in range(QS)]
        kRc = [persist.tile([128, QW], F32, name=f"kRc{i}") for i in range(QS)]
        vac = [persist.tile([128, 4, 66], F32, name=f"vac{i}") for i in range(QS)]
        wo_s = persist.tile([128, DIM], F32R)
        nc.sync.dma_start(out=wo_s, in_=io["wo"][:, :])
        onesb_s = persist.tile([33, 32], F32R)
        nc.sync.dma_start(out=onesb_s, in_=io["onesb"][:, :])
        ep_s = persist.tile([32, 4, 128], F32R)
        nc.sync.dma_start(out=ep_s, in_=io["ep"][:, :, :])
        ubf_s = persist.tile([128, 128], BF16)
        nc.sync.dma_start(out=ubf_s, in_=io["ubf"][:, :])
        ibf_s = persist.tile([128, 128], BF16)
        nc.sync.dma_start(out=ibf_s, in_=io["ibf"][:, :])

        # ---------------- phases A-C: projections + preprocessing ----------
        with (
            tc.tile_pool(name="ph1", bufs=1) as ph1,
            tc.tile_pool(name="xpool", bufs=2) as xpool,
            tc.tile_pool(name="scr", bufs=2) as scr,
            tc.tile_pool(name="psA", bufs=2, space=PSUM) as psA,
            tc.tile_pool(name="psS", bufs=2, space=PSUM) as psS,
        ):
            wq_s = ph1.tile([128, 8, 128], F32R)
            wkd_s = ph1.tile([128, 8, 128], F32R)
            wvg_s = ph1.tile([128, 8, 128], F32R)
            for w_s, name in ((wq_s, "wq"), (wkd_s, "wkd"), (wvg_s, "wvg")):
                nc.sync.dma_start(
                    out=w_s, in_=io[name].ap().rearrange("(k p) m -> p k m", p=128)
                )
            c1_s = ph1.tile([128, S], F32)
            c2_s = ph1.tile([128, S], F32)
            nc.sync.dma_start(out=c1_s, in_=io["c1"][:, :])
            nc.sync.dma_start(out=c2_s, in_=io["c2"][:, :])
            perm_s = ph1.tile([128, 128], F32R)
            nc.sync.dma_start(out=perm_s, in_=io["perm"][:, :])
            ident_s = ph1.tile([64, 64], F32)
            nc.sync.dma_start(out=ident_s, in_=io["ident"][:, :])
            mg_s = ph1.tile([2, 128], F32R)
            nc.sync.dma_start(out=mg_s, in_=io["mg"][:, :])
            mones_s = ph1.tile([1, 128], F32R)
            nc.sync.dma_start(out=mones_s, in_=io["mones"][:, :])
            msq_s = ph1.tile([128, 2], F32R)
            nc.sync.dma_start(out=msq_s, in_=io["msq"][:, :])
            mok_s = ph1.tile([64, 1], F32R)
            nc.sync.dma_start(out=mok_s, in_=io["mok"][:, :])

            eps_t = ph1.tile([128, 1], F32)
            nc.vector.memset(eps_t, EPS)
            ones_t = ph1.tile([128, 1], F32)
            nc.vector.memset(ones_t, 1.0)

            qt = ph1.tile([128, S], F32)
            kt = ph1.tile([128, S], F32)
            vg = ph1.tile([128, S], F32)

            for sc in range(QS):
                sl = slice(QW * sc, QW * sc + QW)
                xt_sc = xpool.tile([128, 8, QW], F32R, tag="xt", name=f"xt{sc}")
                nc.sync.dma_start(
                    out=xt_sc,
                    in_=io["xt"].ap()[:, sl].rearrange("(k p) s -> p k s", p=128),
                )
                # projections: qT, kTd(duplicated), vgT
                for w_s, dst in ((wq_s, qt), (wkd_s, kt), (wvg_s, vg)):
                    acc = psA.tile([128, QW], F32, tag="acc", name=f"acc{sc}")
                    for kc in range(8):
                        nc.tensor.matmul(
                            acc,
                            w_s[:, kc, :],
                            xt_sc[:, kc, :],
                            start=(kc == 0),
                            stop=(kc == 7),
                        )
                    if dst is vg:
                        nc.vector.tensor_copy(dst[:, sl], acc)
                    else:
                        nc.vector.tensor_copy(_r(dst[:, sl]), acc)

                # rms-norm scales: 1/sqrt(sumsq/64 + eps); q-gain via mg
                qsq = scr.tile([128, QW], F32, tag="qsq", name=f"qsq{sc}")
                nc.scalar.activation(
                    _r(qsq), qt[:, sl], mybir.ActivationFunctionType.Square
                )
                ksq = scr.tile([64, QW], F32, tag="ksq", name=f"ksq{sc}")
                nc.scalar.activation(
                    _r(ksq), kt[0:64, sl], mybir.ActivationFunctionType.Square
                )
                sq = psS.tile([2, QW], F32, tag="sq", name=f"sq{sc}")
                nc.tensor.matmul(sq, msq_s, _r(qsq))
                sqq = scr.tile([2, QW], F32, tag="sqq", name=f"sqq{sc}")
                nc.scalar.activation(
                    sqq, sq, mybir.ActivationFunctionType.Sqrt,
                    bias=eps_t[0:2, :], scale=1.0 / HD,
                )
                rsq_q = scr.tile([2, QW], F32, tag="rsq_q", name=f"rsq_q{sc}")
                with nc.allow_low_precision(reason="f32r rounding intentional"):
                    nc.vector.reciprocal(_r(rsq_q), sqq)
                sk = psS.tile([1, QW], F32, tag="sk", name=f"sk{sc}")
                nc.tensor.matmul(sk, mok_s, _r(ksq))
                skq = scr.tile([1, QW], F32, tag="skq", name=f"skq{sc}")
                nc.scalar.activation(
                    skq, sk, mybir.ActivationFunctionType.Sqrt,
                    bias=eps_t[0:1, :], scale=1.0 / HD,
                )
                rsq_k = scr.tile([1, QW], F32, tag="rsq_k", name=f"rsq_k{sc}")
                with nc.allow_low_precision(reason="f32r rounding intentional"):
                    nc.vector.reciprocal(_r(rsq_k), skq)

                # broadcast scales over partitions (PE) + apply in place
                for mask, src_r, tgt in (
                    (mg_s, rsq_q, qt), (mones_s, rsq_k, kt)
                ):
                    bc = psA.tile([128, QW], F32, tag="acc", name=f"bc{sc}")
                    nc.tensor.matmul(bc, mask, _r(src_r))
                    nc.vector.tensor_mul(_r(tgt[:, sl]), tgt[:, sl], bc)

                # rotary (linear, post-scale): swap halves via PE perm matmul
                for src_t, dst in ((qt, qRc[sc]), (kt, kRc[sc])):
                    sw = psA.tile([128, QW], F32, tag="acc", name=f"sw{sc}")
                    nc.tensor.matmul(sw, perm_s, _r(src_t[:, sl]))
                    t1 = scr.tile([128, QW], F32, tag="rot1", name=f"t1_{sc}")
                    nc.vector.tensor_mul(t1, src_t[:, sl], c1_s[:, sl])
                    t2 = scr.tile([128, QW], F32, tag="rot2", name=f"t2_{sc}")
                    nc.vector.tensor_mul(t2, sw, c2_s[:, sl])
                    nc.vector.tensor_add(_r(dst[:, :]), t1, t2)

                # value gate + transpose into v_aug chunk [128, 4, 66]
                nc.vector.tensor_copy(
                    _r(vac[sc][:, :, 32:33]), ones_t.to_broadcast((128, 4, 1))
                )
                nc.vector.tensor_copy(
                    _r(vac[sc][:, :, 65:66]), ones_t.to_broadcast((128, 4, 1))
                )
                sg = scr.tile([64, QW], F32, tag="qsq", name=f"sg{sc}")
                nc.scalar.activation(
                    sg, vg[64:128, sl], mybir.ActivationFunctionType.Sigmoid
                )
                vga = scr.tile([64, QW], F32, tag="ksq", name=f"vga{sc}")
                nc.vector.tensor_mul(vga, vg[0:64, sl], sg)
                for jj in range(4):
                    tv = psA.tile([128, 64], F32, tag="acc", name=f"tv{sc}_{jj}")
                    nc.tensor.transpose(
                        tv, vga[:, 128 * jj:128 * jj + 128], ident_s
                    )
                    dst = vac[sc][:, jj, :].rearrange("p (a b) -> p a b", a=2)[
                        :, :, 0:32
                    ]
                    src = tv.rearrange("p (a b) -> p a b", a=2)
                    nc.vector.tensor_copy(_r(dst), src)

        if DEBUG:
            for i in range(QS):
                nc.sync.dma_start(
                    out=io["dbg_qR"][:, QW * i:QW * i + QW], in_=qRc[i]
                )
                nc.sync.dma_start(
                    out=io["dbg_kR"][:, QW * i:QW * i + QW], in_=kRc[i]
                )
                nc.sync.dma_start(
                    out=io["dbg_va"][:, 264 * i:264 * i + 264],
                    in_=vac[i].rearrange("p a b -> p (a b)"),
                )

        # ---------------- phase D: attention + output projection -----------
        with (
            tc.tile_pool(name="psST", bufs=1, space=PSUM) as psST,
            tc.tile_pool(name="psAT", bufs=4, space=PSUM) as psAT,
            tc.tile_pool(name="ptp", bufs=3) as ptp,
            tc.tile_pool(name="utp", bufs=2) as utp,
            tc.tile_pool(name="smp", bufs=4) as smp,
            tc.tile_pool(name="outp", bufs=3) as outp,
        ):
            for qs in range(QS):
                at = [
                    psAT.tile([33, QW], F32, tag="at", name=f"at{qs}_{su}")
                    for su in range(4)
                ]
                nkb = 4 * qs + 4
                for kb in range(nkb):
                    off = max(0, 128 * (kb - 4 * qs))
                    st = psST.tile([128, 4, QW], F32, tag="st", name=f"st{qs}_{kb}")
                    for su in range(4):
                        rows = slice(32 * su, 32 * su + 32)
                        nc.tensor.matmul(
                            st[:, su, off:QW],
                            _r(kRc[kb // 4][rows, 128 * (kb % 4):128 * (kb % 4) + 128]),
                            _r(qRc[qs][rows, off:QW]),
                            tile_position=(32 * su, 0),
                        )
                    if kb >= 4 * qs:
                        # exact-diagonal block: add -1e9 strictly-lower (k>q)
                        # via a bf16 matmul accumulation (PE-only masking)
                        for su in range(4):
                            nc.tensor.matmul(
                                st[:, su, off:off + 128],
                                ubf_s,
                                ibf_s,
                                start=False,
                                stop=True,
                                skip_group_check=True,
                            )
                    pt = ptp.tile([128, 4, QW], F32, tag="pt", name=f"pt{qs}_{kb}")
                    nc.scalar.activation(
                        _r(pt[:, :, off:QW]), st[:, :, off:QW],
                        mybir.ActivationFunctionType.Exp,
                    )
                    if DEBUG and qs == 0 and kb == 0:
                        nc.sync.dma_start(
                            out=io["dbg_pt"][:, :],
                            in_=pt.rearrange("p a b -> p (a b)"),
                        )
                    for su in range(4):
                        lo = 0 if su % 2 == 0 else 33
                        nc.tensor.matmul(
                            at[su][:, off:QW],
                            _r(vac[kb // 4][:, kb % 4, lo:lo + 33]),
                            _r(pt[:, su, off:QW]),
                            start=(kb == 0),
                            stop=(kb == nkb - 1),
                            skip_group_check=True,
                        )

                # normalize + assemble uT (PE-only partition moves).
                # dbu reuses the score-arena PSUM slot: banks 0/1 rotate as
                # denominator broadcasts, bank 2 is the placement accumulator.
                ut = utp.tile([128, QW], F32R, tag="ut", name=f"ut{qs}")
                dbu = psST.tile([128, 4, QW], F32, tag="st", name=f"dbu{qs}")
                for su in range(4):
                    atc = smp.tile([32, QW], F32, tag="atc", name=f"atc{qs}_{su}")
                    nc.vector.tensor_copy(atc, at[su][0:32, :])
                    rcp = smp.tile([33, QW], F32, tag="rcp", name=f"rcp{qs}_{su}")
                    with nc.allow_low_precision(reason="f32r rounding"):
                        nc.vector.reciprocal(
                            _r(rcp[32:33, :]), at[su][32:33, :]
                        )
                    dbv = dbu[0:32, su % 2, :]
                    nc.tensor.matmul(
                        dbv,
                        onesb_s[32:33, :],
                        _r(rcp[32:33, :]),
                        tile_position=(32, 0),
                    )
                    utmp = smp.tile([32, QW], F32, tag="utmp", name=f"utmp{qs}_{su}")
                    nc.vector.tensor_mul(_r(utmp), atc, dbv)
                    nc.tensor.matmul(
                        dbu[:, 2, :],
                        ep_s[:, su, :],
                        _r(utmp),
                        start=(su == 0),
                        stop=(su == 3),
                        skip_group_check=True,
                    )
                nc.vector.tensor_copy(_r(ut), dbu[:, 2, :])
                if DEBUG:
                    nc.sync.dma_start(
                        out=_r(io["dbg_ut"][:, QW * qs:QW * qs + QW]), in_=ut
                    )

                # output projection for this q-super
                for sb in range(4):
                    ob = outp.tile([128, DIM], F32, tag="ob", name=f"ob{qs}_{sb}")
                    for ncn in range(2):
                        op = psAT.tile(
                            [128, QW], F32, tag="at", name=f"op{qs}_{sb}_{ncn}"
                        )
                        nc.tensor.matmul(
                            op,
                            ut[:, 128 * sb:128 * sb + 128],
                            wo_s[:, QW * ncn:QW * ncn + QW],
                        )
                        nc.vector.tensor_copy(ob[:, QW * ncn:QW * ncn + QW], op)
                    row = QW * qs + 128 * sb
                    nc.sync.dma_start(out=io["out"][row:row + 128, :], in_=ob)


@functools.lru_cache(maxsize=1)
def _build():
    nc = bacc.Bacc(
        "TRN2", target_bir_lowering=False, debug=False, num_devices=NCORES
    )
    io = {
        "xt": nc.dram_tensor("xt", [DIM, S], F32R, kind="ExternalInput"),
        "wq": nc.dram_tensor("wq", [DIM, 128], F32R, kind="ExternalInput"),
        "wkd": nc.dram_tensor("wkd", [DIM, 128], F32R, kind="ExternalInput"),
        "wvg": nc.dram_tensor("wvg", [DIM, 128], F32R, kind="ExternalInput"),
        "wo": nc.dram_tensor("wo", [128, DIM], F32R, kind="ExternalInput"),
        "c1": nc.dram_tensor("c1", [128, S], F32, kind="ExternalInput"),
        "c2": nc.dram_tensor("c2", [128, S], F32, kind="ExternalInput"),
        "perm": nc.dram_tensor("perm", [128, 128], F32R, kind="ExternalInput"),
        "ident": nc.dram_tensor("ident", [64, 64], F32, kind="ExternalInput"),
        "mg": nc.dram_tensor("mg", [2, 128], F32R, kind="ExternalInput"),
        "mones": nc.dram_tensor("mones", [1, 128], F32R, kind="ExternalInput"),
        "msq": nc.dram_tensor("msq", [128, 2], F32R, kind="ExternalInput"),
        "mok": nc.dram_tensor("mok", [64, 1], F32R, kind="ExternalInput"),
        "onesb": nc.dram_tensor("onesb", [33, 32], F32R, kind="ExternalInput"),
        "ep": nc.dram_tensor("ep", [32, 4, 128], F32R, kind="ExternalInput"),
        "ubf": nc.dram_tensor("ubf", [128, 128], BF16, kind="ExternalInput"),
        "ibf": nc.dram_tensor("ibf", [128, 128], BF16, kind="ExternalInput"),
        "out": nc.dram_tensor("out", [S, DIM], F32, kind="ExternalOutput"),
    }
    if DEBUG:
        io["dbg_qR"] = nc.dram_tensor("dbg_qR", [128, S], F32, kind="ExternalOutput")
        io["dbg_kR"] = nc.dram_tensor("dbg_kR", [128, S], F32, kind="ExternalOutput")
        io["dbg_va"] = nc.dram_tensor(
            "dbg_va", [128, 16 * 66], F32, kind="ExternalOutput"
        )
        io["dbg_pt"] = nc.dram_tensor(
            "dbg_pt", [128, 4 * 512], F32, kind="ExternalOutput"
        )
        io["dbg_ut"] = nc.dram_tensor("dbg_ut", [128, S], F32, kind="ExternalOutput")
    with tile.TileContext(nc) as tc:
        _build_kernel(tc, io)
    nc.compile()
    return nc


def _tf32(x):
    """Round f32 array to tfloat32 bit pattern (RNE-ish) so the PE's f32r
    truncation is exact on pre-rounded data."""
    b = np.ascontiguousarray(x, np.float32).view(np.uint32)
    out = ((b + np.uint32(0x00001000)) & np.uint32(0xFFFFE000)).view(np.float32)
    return np.ascontiguousarray(out)


def _host_tables():
    i = np.arange(0, HD, 2, dtype=np.float32) / HD * math.pi  # [32]
    pos = np.arange(S, dtype=np.float32)
    radius = 1.0 / (1.0 + pos[:, None] * 0.01)
    ang = pos[:, None] * i[None, :]
    cosT = np.ascontiguousarray((radius * np.cos(ang)).T.astype(np.float32))
    sinT = np.ascontiguousarray((radius * np.sin(ang)).T.astype(np.float32))
    c1 = np.tile(cosT, (4, 1))
    c2 = np.concatenate([sinT, -sinT, sinT, -sinT], 0)
    perm = np.zeros((128, 128), np.float32)
    for m in range(128):
        perm[(m // 64) * 64 + ((m + 32) % 64), m] = 1.0
    msq = np.zeros((128, 2), np.float32)
    msq[0:64, 0] = 1.0
    msq[64:128, 1] = 1.0
    return c1, c2, perm, msq


def make_in_maps(inputs):
    x = np.asarray(inputs["x"], np.float32)
    Wq = np.asarray(inputs["Wq"], np.float32)
    Wk = np.asarray(inputs["Wk"], np.float32)
    Wv = np.asarray(inputs["Wv"], np.float32)
    Wg = np.asarray(inputs["Wg"], np.float32)
    Wo = np.asarray(inputs["Wo"], np.float32)
    q_gain = np.asarray(inputs["q_gain"], np.float32)
    lam = np.asarray(inputs["lambda_param"], np.float32)

    xT = np.ascontiguousarray(x[0].T)  # [DIM, S]
    c1, c2, perm, msq = _host_tables()
    ident = np.eye(64, dtype=np.float32)
    mones = np.ones((1, 128), np.float32)
    mok = np.ones((64, 1), np.float32)
    onesb = np.zeros((33, 32), np.float32)
    onesb[32] = 1.0
    ep = np.zeros((32, 4, 128), np.float32)
    for su in range(4):
        for k in range(32):
            ep[k, su, 32 * su + k] = 1.0
    ubf = np.triu(np.full((128, 128), -1e9, np.float32), 1).astype(
        ml_dtypes.bfloat16
    )
    ibf = np.eye(128, dtype=np.float32).astype(ml_dtypes.bfloat16)

    in_maps = []
    for c in range(NCORES):
        g = c // 2
        h0, h1 = 2 * c, 2 * c + 1
        Wk_g = Wk[64 * g:64 * g + 64]
        Wv_g = Wv[64 * g:64 * g + 64]
        Wg_g = Wg[64 * g:64 * g + 64]
        mg = np.zeros((2, 128), np.float32)
        mg[0, 0:64] = q_gain[h0] / math.sqrt(HALF)
        mg[1, 64:128] = q_gain[h1] / math.sqrt(HALF)
        woP = np.zeros((128, DIM), np.float32)
        for i, h in enumerate((h0, h1)):
            W1 = Wo[:, 64 * h:64 * h + 32]
            W2 = Wo[:, 64 * h + 32:64 * h + 64]
            woP[64 * i:64 * i + 32] = (W1 + W2).T
            woP[64 * i + 32:64 * i + 64] = (lam[h] * (W2 - W1)).T
        in_maps.append({
            "xt": _tf32(xT),
            "wq": _tf32(Wq[128 * c:128 * c + 128].T),
            "wkd": _tf32(np.concatenate([Wk_g, Wk_g], 0).T),
            "wvg": _tf32(np.concatenate([Wv_g, Wg_g], 0).T),
            "wo": _tf32(woP),
            "c1": c1,
            "c2": c2,
            "perm": perm,
            "ident": ident,
            "mg": _tf32(mg),
            "mones": mones,
            "msq": msq,
            "mok": mok,
            "onesb": onesb,
            "ep": ep,
            "ubf": ubf,
            "ibf": ibf,
        })
    return in_maps


def kernel(**inputs):
    nc = _build()
    in_maps = make_in_maps(inputs)
    res = run_bass_kernel_spmd(nc, in_maps, core_ids=list(range(NCORES)))
    total = np.zeros((S, DIM), np.float32)
    for c in range(NCORES):
        total += res.results[c]["out"]
    return total.reshape(1, S, DIM)
